# revision 1
# baseline (speedup 1.0000x reference)
"""Trainium2 Bass kernel for adjacency-masked multi-head attention.

Problem (fixed shapes): x[4,2048,128], A[2048,2048] int32 0/1, Wq[128,128],
Wkv[256,128], Wp[128,128], bp[128]; out = softmax-attention with mask + resid.

Sharding: 8 cores = (batch b in 0..3) x (query half s in 0..1). Each core
computes K/V for its whole batch (cheap) and attention for its 1024 queries.
A is used as a multiplicative bf16 mask (exp then mask; softmax denominator
comes for free out of the PV matmul via all-ones stationary columns).

Everything is laid out transposed ([channel, token]) so the scores matmuls
run row-tiled (K=32 per head, 4 concurrent tiles) and PV runs with M=64
([v_h | ones]) emitting numerator and denominator in one stream.
"""

import contextlib

import numpy as np

_CACHE = {}

B, N, C, H, HD = 4, 2048, 128, 4, 32
NQ = 1024
SCALE = HD ** -0.5
KB = N // 128
QC = NQ // 512


def _build():
    import concourse.bacc as bacc
    import concourse.mybir as mybir
    import concourse.tile as tile
    from concourse.tile_rust import add_dep_helper

    F32 = mybir.dt.float32
    BF16 = mybir.dt.bfloat16
    EXP = mybir.ActivationFunctionType.Exp
    ADD = mybir.AluOpType.add

    nc = bacc.Bacc("TRN2", target_bir_lowering=False, debug=False)

    xT = nc.dram_tensor("xT", [C, N], BF16, kind="ExternalInput")
    xqT = nc.dram_tensor("xqT", [C, NQ], F32, kind="ExternalInput")
    AT = nc.dram_tensor("AT", [N, NQ], BF16, kind="ExternalInput")
    WqT = nc.dram_tensor("WqT", [C, C], BF16, kind="ExternalInput")
    WkT = nc.dram_tensor("WkT", [C, C], BF16, kind="ExternalInput")
    WvT = nc.dram_tensor("WvT", [C, C], BF16, kind="ExternalInput")
    SEL = nc.dram_tensor("SEL", [C, C], F32, kind="ExternalInput")
    WpT0 = nc.dram_tensor("WpT0", [C, C], BF16, kind="ExternalInput")
    WpT1 = nc.dram_tensor("WpT1", [C, C], BF16, kind="ExternalInput")
    bpT = nc.dram_tensor("bpT", [C, 1], F32, kind="ExternalInput")
    outT = nc.dram_tensor("outT", [C, NQ], F32, kind="ExternalOutput")

    with tile.TileContext(nc) as tc:
        with (
            tc.tile_pool(name="const", bufs=1) as cpool,
            tc.tile_pool(name="data", bufs=1) as dpool,
        ):
            w_q = cpool.tile([C, C], BF16, name="w_q")
            w_k = cpool.tile([C, C], BF16, name="w_k")
            w_v = cpool.tile([C, C], BF16, name="w_v")
            sel = cpool.tile([C, C], F32, name="sel")
            w_p0 = cpool.tile([C, C], BF16, name="w_p0")
            w_p1 = cpool.tile([C, C], BF16, name="w_p1")
            bp_sb = cpool.tile([C, 1], F32, name="bp_sb")
            nc.sync.dma_start(w_q[:], WqT[:])
            nc.sync.dma_start(w_k[:], WkT[:])
            nc.sync.dma_start(w_v[:], WvT[:])
            nc.sync.dma_start(sel[:], SEL[:])
            nc.sync.dma_start(w_p0[:], WpT0[:])
            nc.sync.dma_start(w_p1[:], WpT1[:])
            nc.sync.dma_start(bp_sb[:], bpT[:])

            xT_sb = dpool.tile([C, N], BF16, name="xT_sb")
            xqT_sb = dpool.tile([C, NQ], F32, name="xqT_sb")
            nc.sync.dma_start(xT_sb[:], xT[:])
            nc.sync.dma_start(xqT_sb[:], xqT[:])
            at_sb = []
            for kb in range(KB):
                t = dpool.tile([128, NQ], BF16, name=f"at{kb}")
                nc.sync.dma_start(t[:], AT[kb * 128:(kb + 1) * 128, :])
                at_sb.append(t)

            kT_sb = dpool.tile([C, N], BF16, name="kT_sb")
            qT_sb = dpool.tile([C, NQ], BF16, name="qT_sb")
            vaug_sb = dpool.tile([128, KB * H * 64], BF16, name="vaug_sb")
            nc.gpsimd.memset(vaug_sb[:], 1.0)

            with tc.tile_pool(name="pjps", bufs=2, space="PSUM") as pjps:
                for ch in range(N // 512):
                    ps = pjps.tile([C, 512], F32, name=f"pk{ch}", tag="pj")
                    nc.tensor.matmul(ps[:], w_k[:], xT_sb[:, ch * 512:(ch + 1) * 512])
                    nc.vector.tensor_copy(kT_sb[:, ch * 512:(ch + 1) * 512], ps[:])
                for ch in range(NQ // 512):
                    ps = pjps.tile([C, 512], F32, name=f"pq{ch}", tag="pj")
                    nc.tensor.matmul(ps[:], w_q[:], xT_sb[:, ch * 512:(ch + 1) * 512])
                    nc.vector.tensor_copy(qT_sb[:, ch * 512:(ch + 1) * 512], ps[:])
                for kb in range(KB):
                    ps = pjps.tile([128, C], F32, name=f"pv{kb}", tag="pj")
                    nc.tensor.matmul(
                        ps[:], xT_sb[:, kb * 128:(kb + 1) * 128], w_v[:]
                    )
                    dst = vaug_sb[:, kb * 256:(kb + 1) * 256].rearrange(
                        "p (h x) -> p h x", x=64
                    )[:, :, 0:32]
                    src = ps[:].rearrange("p (h d) -> p h d", d=32)
                    nc.vector.tensor_copy(dst, src)

            with (
                tc.tile_pool(name="sps", bufs=3, space="PSUM") as sps,
                tc.tile_pool(name="accps", bufs=1, space="PSUM") as accps,
                tc.tile_pool(name="ppool", bufs=6) as ppool,
                tc.tile_pool(name="epool", bufs=2) as epool,
            ):
                for qc in range(QC):
                    qs = slice(qc * 512, (qc + 1) * 512)
                    acc_ps = accps.tile([128, 1024], F32, name=f"acc{qc}", tag="acc")
                    last_score_mm = [None]

                    def emit_pv(kb, hp, p_sb, acc_ps=acc_ps, lsm=last_score_mm):
                        for hh in range(2):
                            h = hp * 2 + hh
                            m, b = h % 2, h // 2
                            mm = nc.tensor.matmul(
                                acc_ps[64 * m:64 * (m + 1), b * 512:(b + 1) * 512],
                                vaug_sb[:, kb * 256 + h * 64:kb * 256 + (h + 1) * 64],
                                p_sb[:, hh * 512:(hh + 1) * 512],
                                start=(kb == 0),
                                stop=(kb == KB - 1),
                                tile_position=(0, 64 * m),
                            )
                            if lsm[0] is not None:
                                add_dep_helper(
                                    mm.ins, lsm[0], sync=False,
                                    reason="sw-pipeline PE order",
                                )

                    pending = []
                    for kb in range(KB):
                        ks = slice(kb * 128, (kb + 1) * 128)
                        s_tiles = [
                            sps.tile([128, 1024], F32, name=f"s{qc}_{kb}_{hp}", tag="s")
                            for hp in range(2)
                        ]
                        for h in range(H):
                            hs = slice(32 * h, 32 * (h + 1))
                            mm = nc.tensor.matmul(
                                s_tiles[h // 2][:, (h % 2) * 512:(h % 2 + 1) * 512],
                                kT_sb[hs, ks],
                                qT_sb[hs, qs],
                                tile_position=(32 * h, 0),
                            )
                            last_score_mm[0] = mm.ins
                        while pending:
                            emit_pv(*pending.pop(0))
                        for hp in range(2):
                            s_ps = s_tiles[hp]
                            p_sb = ppool.tile(
                                [128, 1024], BF16, name=f"p{qc}_{kb}_{hp}", tag="p"
                            )
                            nc.scalar.activation(p_sb[:], s_ps[:], EXP)
                            for hh in range(2):
                                nc.vector.tensor_mul(
                                    p_sb[:, hh * 512:(hh + 1) * 512],
                                    p_sb[:, hh * 512:(hh + 1) * 512],
                                    at_sb[kb][:, qs],
                                )
                            pending.append((kb, hp, p_sb))
                    for args_pv in pending:
                        emit_pv(*args_pv)

                    rr_sb = epool.tile([128, 1024], F32, name=f"rr{qc}", tag="rr")
                    nc.vector.reciprocal(rr_sb[:], acc_ps[:])
                    asc_sb = epool.tile([128, 1024], BF16, name=f"asc{qc}", tag="asc")
                    o2 = sps.tile([128, 512], F32, name=f"o2_{qc}", tag="s")
                    for b in range(2):
                        bc_ps = sps.tile([128, 512], F32, name=f"bc{qc}_{b}", tag="s")
                        nc.tensor.matmul(
                            bc_ps[:], sel[:], rr_sb[:, b * 512:(b + 1) * 512]
                        )
                        bc_sb = epool.tile(
                            [128, 512], F32, name=f"bcs{qc}_{b}", tag="bcs"
                        )
                        nc.vector.tensor_copy(bc_sb[:], bc_ps[:])
                        nc.vector.tensor_mul(
                            asc_sb[:, b * 512:(b + 1) * 512],
                            acc_ps[:, b * 512:(b + 1) * 512],
                            bc_sb[:],
                        )
                    for b, w_pb in enumerate((w_p0, w_p1)):
                        nc.tensor.matmul(
                            o2[:],
                            w_pb[:],
                            asc_sb[:, b * 512:(b + 1) * 512],
                            start=(b == 0),
                            stop=(b == 1),
                        )
                    o_sb = epool.tile([128, 512], F32, name=f"ot{qc}", tag="ot")
                    nc.vector.scalar_tensor_tensor(
                        o_sb[:], o2[:], bp_sb[:], xqT_sb[:, qs], ADD, ADD
                    )
                    nc.sync.dma_start(outT[:, qs], o_sb[:])

    nc.compile()
    return nc


def _prep_in_maps(x, A, Wq, Wkv, Wp, bp):
    import ml_dtypes

    bf16 = ml_dtypes.bfloat16
    x = np.asarray(x, np.float32)
    A = np.asarray(A)
    Wq = np.asarray(Wq, np.float32)
    Wkv = np.asarray(Wkv, np.float32)
    Wp = np.asarray(Wp, np.float32)
    bp = np.asarray(bp, np.float32)

    wq = np.ascontiguousarray((Wq * SCALE).T).astype(bf16)
    wk = np.ascontiguousarray(Wkv[:C].T).astype(bf16)
    wv = np.ascontiguousarray(Wkv[C:].T).astype(bf16)
    bpT = np.ascontiguousarray(bp.reshape(C, 1))
    Af = A.astype(np.float32)

    # selector matmul constant: bcast[j, q] = rr[64*(j//64)+32, q]
    selm = np.zeros((C, C), np.float32)
    for j in range(C):
        selm[64 * (j // 64) + 32, j] = 1.0
    # Wp.T rows rearranged to the PV accumulator layout (denominator rows = 0)
    wpT = Wp.T
    wpb = []
    for b in range(2):
        w = np.zeros((C, C), np.float32)
        for r in range(C):
            d = r % 64
            if d < 32:
                w[r, :] = wpT[32 * (2 * b + r // 64) + d, :]
        wpb.append(np.ascontiguousarray(w).astype(bf16))

    in_maps = []
    for core in range(8):
        b, s = divmod(core, 2)
        sl = slice(s * NQ, (s + 1) * NQ)
        xTb = np.ascontiguousarray(x[b].T)
        in_maps.append(
            {
                "xT": xTb.astype(bf16),
                "xqT": np.ascontiguousarray(xTb[:, sl]),
                "AT": np.ascontiguousarray(Af[sl, :].T).astype(bf16),
                "WqT": wq,
                "WkT": wk,
                "WvT": wv,
                "SEL": selm,
                "WpT0": wpb[0],
                "WpT1": wpb[1],
                "bpT": bpT,
            }
        )
    return in_maps


def kernel(x, A, Wq, Wkv, Wp, bp):
    from concourse.bass_utils import run_bass_kernel_spmd

    if "nc" not in _CACHE:
        _CACHE["nc"] = _build()
    nc = _CACHE["nc"]
    in_maps = _prep_in_maps(x, A, Wq, Wkv, Wp, bp)
    res = run_bass_kernel_spmd(nc, in_maps, list(range(8)))
    out = np.empty((B, N, C), np.float32)
    for core in range(8):
        b, s = divmod(core, 2)
        out[b, s * NQ:(s + 1) * NQ, :] = res.results[core]["outT"].T
    return out
